# revision 6
# baseline (speedup 1.0000x reference)
"""AttentiveDensenet Trainium2 Bass kernel.

Data-parallel over batch B=8 across 8 NeuronCores (1 image per core).
Per layer l (of 4):
  - Q/K/V 1x1 convs as bf16 matmuls with x-tiles as the stationary operand,
    producing position-major [pos, (head, dim)] activations directly.
    Q is computed first so score products (vs. prior layers' keys) overlap
    the K/V matmuls. Bias via a K=1 ones-row matmul accumulated into PSUM.
  - Scores/attn are stored t-major [pos, t, group]; score d-reduction is a
    3-step bf16 halving tree (DVE 2x) + small reduce. V/O use a d-major
    channel order (host-side permutation) so the weighted-sum broadcast is
    dense. Top-k threshold (2nd smallest of 5) via a 10-op min/max network.
  - The weighted sum runs in pb-halves: once half A is transposed
    (PE identity-matmul through PSUM, 4 pb per bank) into the padded conv
    input, conv1's first row-chunk (chunk-outer order) runs on the PE
    while DVE finishes half B.
  - conv3x3 #1 as 9 shifted 1x1 convs accumulated in PSUM (bf16); per-layer
    weights preloaded into SBUF with one contiguous DMA. BN partial stats
    are computed per (chunk, co) as conv1 output lands; one AllReduce per
    layer. A dummy warmup AllReduce at kernel start absorbs first-sync
    skew off the critical path.
  - h1 = relu(A*y1 + B) on ACT (row-split to unblock conv2 chunk 1).
  - conv3x3 #2 (bf16, ob2 bias as a ones-row matmul) + residual
    x += gamma*h2' fused via scalar_tensor_tensor from PSUM.
"""
import numpy as np
import ml_dtypes

import concourse.bacc as bacc
import concourse.mybir as mybir
import concourse.tile as tile
from concourse import bass_utils

L, C, B, H, W = 4, 256, 8, 32, 32
NH, KD = 8, 64
KH = NH * KD          # 512
HW = H * W            # 1024
P = 128
NC = 8                # cores
TOPK = 4
EPS = 1e-7
BN_EPS = 1e-5
PW = W + 2            # 34
PHW = PW * (H + 2)    # 1156

f32 = mybir.dt.float32
bf16 = mybir.dt.bfloat16
AX = mybir.AxisListType
OP = mybir.AluOpType
ACTF = mybir.ActivationFunctionType

_compiled = {}


def _build(ncores=NC, no_cc=False, layers=L, stages=99):
    nc = bacc.Bacc(None, target_bir_lowering=False, debug=False, num_devices=ncores)

    # ---- DRAM I/O (per-core shapes) ----
    xin = nc.dram_tensor("xin", [C, HW], f32, kind="ExternalInput").ap()
    wq = nc.dram_tensor("wq", [L, 2, P, KH], bf16, kind="ExternalInput").ap()
    wk = nc.dram_tensor("wk", [L, 2, P, KH], bf16, kind="ExternalInput").ap()
    wv = nc.dram_tensor("wv", [L, 2, P, KH], bf16, kind="ExternalInput").ap()
    bq = nc.dram_tensor("bq", [L, 1, KH], bf16, kind="ExternalInput").ap()
    bk = nc.dram_tensor("bk", [L, 1, KH], bf16, kind="ExternalInput").ap()
    bv = nc.dram_tensor("bv", [L, 1, KH], bf16, kind="ExternalInput").ap()
    # conv weights packed partition-major: [L, cin128, tap*ci*co, cout128]
    w1d = nc.dram_tensor("w1d", [L, P, 72 * P], bf16, kind="ExternalInput").ap()
    w2d = nc.dram_tensor("w2d", [L, P, 36 * P], bf16, kind="ExternalInput").ap()
    ob2bd = nc.dram_tensor("ob2bd", [L, 2, 1, P], bf16, kind="ExternalInput").ap()
    ident_d = nc.dram_tensor("ident", [P, P], bf16, kind="ExternalInput").ap()
    bng2d = nc.dram_tensor("bng2d", [L, P, 2], f32, kind="ExternalInput").ap()
    bnb2d = nc.dram_tensor("bnb2d", [L, P, 2], f32, kind="ExternalInput").ap()
    gamd = nc.dram_tensor("gamd", [L, P, 1], f32, kind="ExternalInput").ap()
    out = nc.dram_tensor("out", [C, HW], f32, kind="ExternalOutput").ap()

    with tile.TileContext(nc) as tc:
        with tc.tile_pool(name="main", bufs=1) as mp, \
             tc.tile_pool(name="prodp", bufs=2) as prodp, \
             tc.tile_pool(name="tmpp", bufs=2) as tmpp, \
             tc.tile_pool(name="wkvp", bufs=4) as wkvp, \
             tc.tile_pool(name="biasp", bufs=3) as biasp, \
             tc.tile_pool(name="kqvps", bufs=3, space="PSUM") as kqvps, \
             tc.tile_pool(name="convps", bufs=3, space="PSUM") as convps, \
             tc.tile_pool(name="trps", bufs=2, space="PSUM") as trps, \
             tc.tile_pool(name="dramp", bufs=2, space="DRAM") as dramp:

            # persistent tiles
            x = [mp.tile([P, HW], f32, name=f"x{i}") for i in range(2)]
            xb = [mp.tile([P, HW], bf16, name=f"xb{i}") for i in range(2)]
            qbt = mp.tile([P, 8 * KH], bf16, name="qbt")
            kbt = [mp.tile([P, 8 * KH], bf16, name=f"kbt{i}") for i in range(L)]
            vbt = [mp.tile([P, 8 * KH], bf16, name=f"vbt{i}") for i in range(L)]
            S = mp.tile([P, 5 * 64], f32, name="S")          # [p, t, grp]
            attn = mp.tile([P, 5 * 64], f32, name="attn")    # [p, t, grp]
            attnb = mp.tile([P, 5 * 64], bf16, name="attnb")
            mx = mp.tile([P, 64], f32, name="mx")
            zs = mp.tile([P, 64], f32, name="zs")
            dmin = mp.tile([P, 64], f32, name="dmin")
            mxp = mp.tile([P, 64], f32, name="mxp")
            tk = mp.tile([P, 256], f32, name="tk")
            o = mp.tile([P, 8 * KH], bf16, name="o")
            opad = [mp.tile([P, PHW + 2], bf16, name=f"opad{i}") for i in range(4)]
            y1 = [mp.tile([P, HW], f32, name=f"y1_{i}") for i in range(2)]
            sqbuf = mp.tile([P, HW], f32, name="sqbuf")
            h1p = [mp.tile([P, PHW + 2], bf16, name=f"h1p{i}") for i in range(2)]
            stc = mp.tile([P, 12], f32, name="stc")  # [(stat,co), chunk(3)]
            st = mp.tile([P, 4], f32, name="st")     # [sum0,sum1,sq0,sq1]
            gsum = mp.tile([P, 4], f32, name="gsum")
            ones1 = mp.tile([1, P], bf16, name="ones1")
            ones5 = mp.tile([1, 512], bf16, name="ones5")
            ident = mp.tile([P, P], bf16, name="ident")
            # per-layer conv weights (single-buffered; DMA overlaps attention)
            w1s = mp.tile([P, 72 * P], bf16, name="w1s")
            w2s = mp.tile([P, 36 * P], bf16, name="w2s")
            # per-layer consts (reloaded each layer)
            bng2 = mp.tile([P, 2], f32, name="bng2")
            bnb2 = mp.tile([P, 2], f32, name="bnb2")
            ob2bt = [mp.tile([1, P], bf16, name=f"ob2bt{i}") for i in range(2)]
            gamt = mp.tile([P, 1], f32, name="gamt")
            # BN scratch (both co at once)
            t1p = mp.tile([P, 2], f32, name="t1p")
            vartp = mp.tile([P, 2], f32, name="vartp")
            sqp = mp.tile([P, 2], f32, name="sqp")
            stdtp = mp.tile([P, 2], f32, name="stdtp")
            Acp = mp.tile([P, 2], f32, name="Acp")
            Bcp = mp.tile([P, 2], f32, name="Bcp")

            # init
            for i in range(2):
                nc.sync.dma_start(x[i][:], xin[i * P:(i + 1) * P, :])
                nc.scalar.copy(xb[i][:], x[i][:])
            for i in range(4):
                nc.vector.memset(opad[i][:], 0)
            for i in range(2):
                nc.vector.memset(h1p[i][:], 0)
            nc.vector.memset(ones1[:], 1.0)
            nc.vector.memset(ones5[:], 1.0)
            nc.sync.dma_start(ident[:], ident_d)
            nc.vector.memset(S[:], 0)
            nc.vector.memset(attn[:], 0)
            if not no_cc:
                # warmup collective: absorbs first-sync init/skew off-path
                nc.vector.memset(stc[:], 0)
                wui = dramp.tile([1, 4], f32, name="wui")
                wuo = dramp.tile([1, 4], f32, name="wuo", addr_space="Shared")
                nc.sync.dma_start(wui[0], stc[0, 0:4])
                nc.gpsimd.collective_compute(
                    "AllReduce", OP.add, replica_groups=[list(range(ncores))],
                    ins=[wui.opt()], outs=[wuo.opt()])

            S3t = S[:].rearrange("p (t g) -> p t g", g=64)
            at3t = attn[:].rearrange("p (t g) -> p t g", g=64)

            for l in range(layers):
                R = l + 1      # number of real keys
                T = R + 1      # +1 zero key

                # ---- per-layer consts + conv weight preload ----
                nc.sync.dma_start(w1s[:], w1d[l])
                nc.sync.dma_start(w2s[:], w2d[l])
                nc.sync.dma_start(bng2[:], bng2d[l])
                nc.sync.dma_start(bnb2[:], bnb2d[l])
                for i in range(2):
                    nc.sync.dma_start(ob2bt[i][:], ob2bd[l, i])
                nc.sync.dma_start(gamt[:], gamd[l])

                # ---- Q/K/V 1x1 convs, position-major (q first) ----
                for name, wdr, bdr, dest in (
                    ("q", wq, bq, qbt[:]),
                    ("k", wk, bk, kbt[l][:]),
                    ("v", wv, bv, vbt[l][:]),
                ):
                    bt = biasp.tile([1, KH], bf16, name=f"bias_{name}_{l}", tag="bias")
                    nc.sync.dma_start(bt[:], bdr[l])
                    wts = []
                    for ct in range(2):
                        wt = wkvp.tile([P, KH], bf16, name=f"w_{name}_{l}_{ct}", tag="wkv")
                        nc.sync.dma_start(wt[:], wdr[l, ct])
                        wts.append(wt)
                    for pb in range(8):
                        ps = kqvps.tile([P, KH], f32, name="kqv_ps")
                        nc.tensor.matmul(ps[:], ones1[:], bt[:], start=True, stop=False)
                        nc.tensor.matmul(ps[:], xb[0][:, pb * P:(pb + 1) * P], wts[0][:],
                                         start=False, stop=False)
                        nc.tensor.matmul(ps[:], xb[1][:, pb * P:(pb + 1) * P], wts[1][:],
                                         start=False, stop=True)
                        nc.scalar.copy(dest[:, pb * KH:(pb + 1) * KH], ps[:])

                # ---- scores: products + 3-step halving tree + reduce ----
                if stages < 2: continue
                for t in range(R):
                    pr = prodp.tile([P, 8 * KH], bf16, name="prodb")
                    pr3 = pr[:].rearrange("p (g d) -> p g d", d=KD)
                    nc.vector.tensor_mul(pr[:], qbt[:], kbt[t][:])
                    nc.vector.tensor_add(pr3[:, :, 0:32], pr3[:, :, 0:32],
                                         pr3[:, :, 32:64])
                    nc.vector.tensor_add(pr3[:, :, 0:16], pr3[:, :, 0:16],
                                         pr3[:, :, 16:32])
                    nc.vector.tensor_add(pr3[:, :, 0:8], pr3[:, :, 0:8],
                                         pr3[:, :, 8:16])
                    nc.vector.tensor_reduce(
                        out=S[:, t * 64:(t + 1) * 64], in_=pr3[:, :, 0:8],
                        axis=AX.X, op=OP.add)
                nc.vector.memset(S[:, R * 64:(R + 1) * 64], 0)  # zero key

                # ---- softmax over T slots (t-major; dense broadcasts) ----
                if stages < 3: continue
                nc.vector.tensor_reduce(
                    out=mx[:], in_=S[:, 0:T * 64].rearrange("p (t g) -> p g t", g=64),
                    axis=AX.X, op=OP.max)
                nc.vector.tensor_tensor(
                    at3t[:, 0:T, :], S3t[:, 0:T, :],
                    mx[:].unsqueeze(1).broadcast_to([P, T, 64]), OP.subtract)
                nc.scalar.activation(attn[:, 0:T * 64], attn[:, 0:T * 64], ACTF.Exp)
                nc.vector.tensor_reduce(
                    out=zs[:], in_=attn[:, 0:T * 64].rearrange("p (t g) -> p g t", g=64),
                    axis=AX.X, op=OP.add)
                nc.vector.reciprocal(zs[:], zs[:])
                nc.vector.tensor_tensor(
                    at3t[:, 0:T, :], at3t[:, 0:T, :],
                    zs[:].unsqueeze(1).broadcast_to([P, T, 64]), OP.mult)

                # ---- sparse top-k (only T=5): 2nd smallest of 5 ----
                if T > TOPK:
                    a = [at3t[:, i, :] for i in range(5)]
                    tk4 = tk[:].rearrange("p (i g) -> p i g", g=64)
                    nc.vector.tensor_tensor(tk4[:, 0], a[0], a[1], OP.min)   # b0
                    nc.vector.tensor_tensor(tk4[:, 1], a[0], a[1], OP.max)   # b1
                    nc.vector.tensor_tensor(tk4[:, 2], a[2], a[3], OP.min)   # b2
                    nc.vector.tensor_tensor(tk4[:, 3], a[2], a[3], OP.max)   # b3
                    nc.vector.tensor_tensor(mx[:], tk4[:, 0], tk4[:, 2], OP.min)  # c0
                    nc.vector.tensor_tensor(mxp[:], tk4[:, 0], tk4[:, 2], OP.max)  # c1
                    nc.vector.tensor_tensor(zs[:], tk4[:, 1], tk4[:, 3], OP.min)   # d
                    nc.vector.tensor_tensor(mxp[:], mxp[:], zs[:], OP.min)   # s4
                    nc.vector.tensor_tensor(mx[:], mx[:], a[4], OP.max)      # e
                    nc.vector.tensor_tensor(dmin[:], mx[:], mxp[:], OP.min)  # delta
                    nc.vector.tensor_scalar_add(dmin[:], dmin[:], EPS)
                    nc.vector.tensor_tensor(
                        at3t[:, 0:T, :], at3t[:, 0:T, :],
                        dmin[:].unsqueeze(1).broadcast_to([P, T, 64]), OP.subtract)
                    nc.vector.tensor_scalar_max(attn[:, 0:T * 64], attn[:, 0:T * 64], 0.0)
                    nc.vector.tensor_reduce(
                        out=zs[:],
                        in_=attn[:, 0:T * 64].rearrange("p (t g) -> p g t", g=64),
                        axis=AX.X, op=OP.add)
                    nc.vector.tensor_scalar_add(zs[:], zs[:], EPS)
                    nc.vector.reciprocal(zs[:], zs[:])
                    nc.vector.tensor_tensor(
                        at3t[:, 0:T, :], at3t[:, 0:T, :],
                        zs[:].unsqueeze(1).broadcast_to([P, T, 64]), OP.mult)

                nc.vector.tensor_copy(attnb[:], attn[:])

                # ---- weighted sum in pb-halves (bf16, d-major) ----
                if stages < 4: continue
                for half in range(2):
                    csl = slice(half * 4 * KH, (half + 1) * 4 * KH)
                    o4 = o[:, csl].rearrange("p (b d g) -> p b d g", d=KD, g=8)
                    for t in range(R):
                        v4 = vbt[t][:, csl].rearrange("p (b d g) -> p b d g",
                                                      d=KD, g=8)
                        ab = attnb[:, t * 64 + half * 32:t * 64 + half * 32 + 32
                                   ].rearrange("p (b g) -> p b g", g=8
                                               ).unsqueeze(2).broadcast_to(
                                       [P, 4, KD, 8])
                        if t == 0:
                            nc.vector.tensor_tensor(o4, v4, ab, OP.mult)
                        else:
                            tm = tmpp.tile([P, 4 * KH], bf16, name="wtmp")
                            tm4 = tm[:].rearrange("p (b d g) -> p b d g", d=KD, g=8)
                            nc.vector.tensor_tensor(tm4, v4, ab, OP.mult)
                            nc.vector.tensor_add(o[:, csl], o[:, csl], tm[:])

                # ---- transposes + conv1, interleaved so chunk 1 runs while
                #      DVE finishes wsum half B ----
                if stages < 5: continue
                CHUNKS = [(0, 15), (15, 15), (30, 2)]

                def transposes(half):
                    for ht in range(4):
                        pst = trps.tile([P, 512], bf16, name="pst")
                        for k in range(4):
                            pb = half * 4 + k
                            nc.tensor.transpose(
                                pst[:, k * P:(k + 1) * P],
                                o[:, (pb * 4 + ht) * P:(pb * 4 + ht + 1) * P],
                                ident[:])
                        opv = opad[ht][:, 0:PHW].rearrange("c (i j) -> c i j", j=PW)
                        dst = opv[:, 1 + 16 * half:17 + 16 * half, 1:W + 1]
                        src = pst[:].rearrange("c (r j) -> c r j", j=W)
                        if (ht + half) % 2 == 0:
                            nc.vector.tensor_copy(dst, src)
                        else:
                            nc.scalar.copy(dst, src)

                def conv1_chunk(ic, i0, nr):
                    for co in range(2):
                        ps = convps.tile([P, 512], f32, name="c1ps", tag="cps")
                        nw = PW * nr
                        for tap in range(9):
                            ty, tx = tap // 3, tap % 3
                            for ci in range(4):
                                wi = (tap * 4 + ci) * 2 + co
                                base = PW * (i0 + ty) + tx
                                nc.tensor.matmul(
                                    ps[:, 0:nw], w1s[:, wi * P:(wi + 1) * P],
                                    opad[ci][:, base:base + nw],
                                    start=(tap == 0 and ci == 0),
                                    stop=(tap == 8 and ci == 3))
                        ych = y1[co][:, W * i0:W * (i0 + nr)]
                        nc.scalar.copy(
                            ych.rearrange("c (i j) -> c i j", j=W),
                            ps[:, 0:nw].rearrange("c (i j) -> c i j", j=PW)[:, :, 0:W])
                        # partial stats for this (chunk, co)
                        nc.vector.tensor_reduce(out=stc[:, co * 3 + ic:co * 3 + ic + 1],
                                                in_=ych, axis=AX.X, op=OP.add)
                        sch = sqbuf[:, W * i0:W * (i0 + nr)]
                        nc.scalar.square(sch, ych)
                        nc.vector.tensor_reduce(
                            out=stc[:, 6 + co * 3 + ic:7 + co * 3 + ic],
                            in_=sch, axis=AX.X, op=OP.add)

                transposes(0)
                conv1_chunk(0, *CHUNKS[0])
                transposes(1)
                conv1_chunk(1, *CHUNKS[1])
                conv1_chunk(2, *CHUNKS[2])

                # ---- BN stats combine + single AllReduce ----
                if stages < 7: continue
                nc.vector.tensor_reduce(
                    out=st[:], in_=stc[:].rearrange("p (q c) -> p q c", c=3),
                    axis=AX.X, op=OP.add)
                if no_cc:
                    nc.vector.tensor_scalar_mul(gsum[:], st[:], float(ncores))
                else:
                    cci = dramp.tile([1, 512], f32, name="cci", tag="cci")
                    cco = dramp.tile([1, 512], f32, name="cco", tag="cco",
                                     addr_space="Shared")
                    nc.sync.dma_start(cci[0].rearrange("(p j) -> p j", j=4), st[:])
                    nc.gpsimd.collective_compute(
                        "AllReduce", OP.add,
                        replica_groups=[list(range(ncores))],
                        ins=[cci.opt()], outs=[cco.opt()])
                    nc.sync.dma_start(gsum[:], cco[0].rearrange("(p j) -> p j", j=4))

                # ---- BN coefficients: A = g/sqrt(var+eps), B = b - mean*A ----
                if stages < 8: continue
                NTOT = float(ncores * HW)
                nc.vector.tensor_scalar_mul(t1p[:], gsum[:, 0:2], 1.0 / NTOT)
                nc.vector.tensor_scalar_mul(vartp[:], gsum[:, 2:4], 1.0 / NTOT)
                nc.vector.tensor_mul(sqp[:], t1p[:], t1p[:])
                nc.vector.tensor_sub(vartp[:], vartp[:], sqp[:])
                nc.vector.tensor_scalar_add(vartp[:], vartp[:], BN_EPS)
                nc.scalar.activation(stdtp[:], vartp[:], ACTF.Sqrt)
                nc.vector.reciprocal(stdtp[:], stdtp[:])
                nc.vector.tensor_mul(Acp[:], bng2[:], stdtp[:])
                nc.vector.tensor_mul(sqp[:], t1p[:], Acp[:])
                nc.vector.tensor_sub(Bcp[:], bnb2[:], sqp[:])
                # h1 = relu(A*y1 + B) (row-split to unblock conv2 chunk 1)
                for (r0, nr2) in ((0, 17), (17, 15)):
                    for co in range(2):
                        h1v = h1p[co][:, 0:PHW].rearrange("c (i j) -> c i j", j=PW)
                        nc.scalar.activation(
                            h1v[:, 1 + r0:1 + r0 + nr2, 1:W + 1],
                            y1[co][:, W * r0:W * (r0 + nr2)].rearrange(
                                "c (i j) -> c i j", j=W),
                            ACTF.Relu, bias=Bcp[:, co:co + 1], scale=Acp[:, co:co + 1])

                # ---- conv3x3 #2 (bf16) + ob2 bias row + residual update ----
                if stages < 9: continue
                for co in range(2):
                    for (i0, nr) in CHUNKS:
                        ps = convps.tile([P, 512], f32, name="c2ps", tag="cps")
                        nw = PW * nr
                        nc.tensor.matmul(ps[:, 0:nw], ob2bt[co][:], ones5[:, 0:nw],
                                         start=True, stop=False)
                        for tap in range(9):
                            ty, tx = tap // 3, tap % 3
                            for ci in range(2):
                                wi = (tap * 2 + ci) * 2 + co
                                base = PW * (i0 + ty) + tx
                                nc.tensor.matmul(
                                    ps[:, 0:nw], w2s[:, wi * P:(wi + 1) * P],
                                    h1p[ci][:, base:base + nw],
                                    start=False, stop=(tap == 8 and ci == 1))
                        xslice = x[co][:, W * i0:W * (i0 + nr)]
                        nc.vector.scalar_tensor_tensor(
                            out=xslice.rearrange("c (i j) -> c i j", j=W),
                            in0=ps[:, 0:nw].rearrange("c (i j) -> c i j", j=PW)[:, :, 0:W],
                            scalar=gamt[:],
                            in1=xslice.rearrange("c (i j) -> c i j", j=W),
                            op0=OP.mult, op1=OP.add)
                    if l < layers - 1:
                        nc.scalar.copy(xb[co][:], x[co][:])
                    else:
                        nc.sync.dma_start(out[co * P:(co + 1) * P, :], x[co][:])

    nc.compile()
    return nc


def _host_prep(inputs):
    bf = ml_dtypes.bfloat16
    kw, kb, qw, qb = inputs["kw"], inputs["kb"], inputs["qw"], inputs["qb"]
    vw, vb = inputs["vw"], inputs["vb"]
    ow1, ow2 = inputs["ow1"], inputs["ow2"]
    gammas, ob2 = inputs["gammas"], inputs["ob2"]

    # d-major channel permutation for V / O / conv1-input
    perm2 = np.arange(512).reshape(NH, KD).T.flatten()  # new (d,g) <- old (g,d)
    vw = vw[:, perm2, :]
    vb = vb[:, perm2]
    ow1 = ow1[:, :, perm2, :, :]

    def packw(wm):  # [L, KH, C] -> [L, 2, 128, KH]
        return np.ascontiguousarray(
            wm.transpose(0, 2, 1).reshape(L, 2, P, KH)).astype(bf)

    d = {}
    d["wq"] = packw(qw / 8.0)
    d["wk"] = packw(kw)
    d["wv"] = packw(vw)
    d["bq"] = np.ascontiguousarray((qb / 8.0).reshape(L, 1, KH)).astype(bf)
    d["bk"] = np.ascontiguousarray(kb.reshape(L, 1, KH)).astype(bf)
    d["bv"] = np.ascontiguousarray(vb.reshape(L, 1, KH)).astype(bf)
    # ow1 [L, 256, 512, 3, 3] -> [L, tap, ci(4), co(2), a(cin128), b(cout128)]
    # -> partition-major [L, a, tap, ci, co, b] for one contiguous DMA/layer
    a1 = ow1.reshape(L, 2, P, 4, P, 3, 3).transpose(0, 5, 6, 3, 1, 4, 2)
    a1 = a1.reshape(L, 9, 4, 2, P, P).transpose(0, 4, 1, 2, 3, 5)
    d["w1d"] = np.ascontiguousarray(a1.reshape(L, P, 72 * P)).astype(bf)
    a2 = ow2.reshape(L, 2, P, 2, P, 3, 3).transpose(0, 5, 6, 3, 1, 4, 2)
    a2 = a2.reshape(L, 9, 2, 2, P, P).transpose(0, 4, 1, 2, 3, 5)
    d["w2d"] = np.ascontiguousarray(a2.reshape(L, P, 36 * P)).astype(bf)
    d["ob2bd"] = np.ascontiguousarray(ob2.reshape(L, 2, 1, P)).astype(bf)
    d["ident"] = np.eye(P, dtype=np.float32).astype(bf)
    d["bng2d"] = np.ascontiguousarray(
        inputs["bn_g"].reshape(L, 2, P).transpose(0, 2, 1)).astype(np.float32)
    d["bnb2d"] = np.ascontiguousarray(
        inputs["bn_b"].reshape(L, 2, P).transpose(0, 2, 1)).astype(np.float32)
    d["gamd"] = np.ascontiguousarray(
        np.broadcast_to(gammas[:, None, None], (L, P, 1))).astype(np.float32)
    return d


def kernel(**inputs):
    if "nc" not in _compiled:
        _compiled["nc"] = _build()
    nc = _compiled["nc"]
    shared = _host_prep(inputs)
    x = np.ascontiguousarray(inputs["x"].reshape(B, C, HW)).astype(np.float32)
    in_maps = []
    for c in range(NC):
        m = dict(shared)
        m["xin"] = x[c]
        in_maps.append(m)
    res = bass_utils.run_bass_kernel_spmd(nc, in_maps, core_ids=list(range(NC)))
    outs = np.stack([res.results[c]["out"] for c in range(NC)])
    return outs.reshape(B, C, H, W).astype(np.float32)


# revision 19
# speedup vs baseline: 1.1176x; 1.1176x over previous
"""AttentiveDensenet Trainium2 Bass kernel.

Data-parallel over batch B=8 across 8 NeuronCores (1 image per core).
Per layer l (of 4):
  - Q/K/V 1x1 convs as bf16 matmuls with x-tiles as the stationary operand,
    producing position-major [pos, (head, dim)] activations directly.
    Q is computed first so score products (vs. prior layers' keys) overlap
    the K/V matmuls. Bias via a K=1 ones-row matmul accumulated into PSUM.
  - Scores/attn are stored t-major [pos, t, group]; score d-reduction is a
    3-step bf16 halving tree (DVE 2x) + small reduce. V/O use a d-major
    channel order (host-side permutation) so the weighted-sum broadcast is
    dense. Top-k threshold (2nd smallest of 5) via a 10-op min/max network.
  - The weighted sum runs in pb-halves: once half A is transposed
    (PE identity-matmul through PSUM, 4 pb per bank) into the padded conv
    input, conv1's first row-chunk (chunk-outer order) runs on the PE
    while DVE finishes half B.
  - conv3x3 #1 as 9 shifted 1x1 convs accumulated in PSUM (bf16); per-layer
    weights preloaded into SBUF with one contiguous DMA. BN partial stats
    are computed per (chunk, co) as conv1 output lands; one AllReduce per
    layer. A dummy warmup AllReduce at kernel start absorbs first-sync
    skew off the critical path.
  - h1 = relu(A*y1 + B) on ACT (row-split to unblock conv2 chunk 1).
  - conv3x3 #2 (bf16, ob2 bias as a ones-row matmul) + residual
    x += gamma*h2' fused via scalar_tensor_tensor from PSUM.
"""
import numpy as np
import ml_dtypes

import concourse.bacc as bacc
import concourse.mybir as mybir
import concourse.tile as tile
from concourse import bass_utils

L, C, B, H, W = 4, 256, 8, 32, 32
NH, KD = 8, 64
KH = NH * KD          # 512
HW = H * W            # 1024
P = 128
NC = 8                # cores
TOPK = 4
EPS = 1e-7
BN_EPS = 1e-5
PW = W + 2            # 34
PHW = PW * (H + 2)    # 1156

f32 = mybir.dt.float32
bf16 = mybir.dt.bfloat16
AX = mybir.AxisListType
OP = mybir.AluOpType
ACTF = mybir.ActivationFunctionType

_compiled = {}


def _build(ncores=NC, no_cc=False, layers=L, stages=99):
    nc = bacc.Bacc(None, target_bir_lowering=False, debug=False, num_devices=ncores)

    # ---- DRAM I/O (per-core shapes) ----
    xin = nc.dram_tensor("xin", [C, HW], f32, kind="ExternalInput").ap()
    wq = nc.dram_tensor("wq", [L, 2, P, KH], bf16, kind="ExternalInput").ap()
    wk = nc.dram_tensor("wk", [L, 2, P, KH], bf16, kind="ExternalInput").ap()
    wv = nc.dram_tensor("wv", [L, 2, P, KH], bf16, kind="ExternalInput").ap()
    bq = nc.dram_tensor("bq", [L, 1, KH], bf16, kind="ExternalInput").ap()
    bk = nc.dram_tensor("bk", [L, 1, KH], bf16, kind="ExternalInput").ap()
    bv = nc.dram_tensor("bv", [L, 1, KH], bf16, kind="ExternalInput").ap()
    # conv weights packed partition-major: [L, cin128, tap*ci*co, cout128]
    w1d = nc.dram_tensor("w1d", [L, P, 72 * P], bf16, kind="ExternalInput").ap()
    w2d = nc.dram_tensor("w2d", [L, P, 36 * P], bf16, kind="ExternalInput").ap()
    ob2bd = nc.dram_tensor("ob2bd", [L, 2, 1, P], bf16, kind="ExternalInput").ap()
    ident_d = nc.dram_tensor("ident", [P, P], bf16, kind="ExternalInput").ap()
    bng2d = nc.dram_tensor("bng2d", [L, P, 2], f32, kind="ExternalInput").ap()
    bnb2d = nc.dram_tensor("bnb2d", [L, P, 2], f32, kind="ExternalInput").ap()
    gamd = nc.dram_tensor("gamd", [L, P, 1], f32, kind="ExternalInput").ap()
    out = nc.dram_tensor("out", [C, HW], f32, kind="ExternalOutput").ap()

    with tile.TileContext(nc) as tc:
        with tc.tile_pool(name="main", bufs=1) as mp, \
             tc.tile_pool(name="prodp", bufs=2) as prodp, \
             tc.tile_pool(name="tmpp", bufs=2) as tmpp, \
             tc.tile_pool(name="wkvp", bufs=4) as wkvp, \
             tc.tile_pool(name="biasp", bufs=3) as biasp, \
             tc.tile_pool(name="kqvps", bufs=3, space="PSUM") as kqvps, \
             tc.tile_pool(name="convps", bufs=3, space="PSUM") as convps, \
             tc.tile_pool(name="trps", bufs=2, space="PSUM") as trps, \
             tc.tile_pool(name="dramp", bufs=2, space="DRAM") as dramp:

            # persistent tiles
            x = [mp.tile([P, HW], f32, name=f"x{i}") for i in range(2)]
            xb = [mp.tile([P, HW], bf16, name=f"xb{i}") for i in range(2)]
            qbt = mp.tile([P, 8 * KH], bf16, name="qbt")
            kbt = [mp.tile([P, 8 * KH], bf16, name=f"kbt{i}") for i in range(L)]
            vbt = [mp.tile([P, 8 * KH], bf16, name=f"vbt{i}") for i in range(L)]
            S = mp.tile([P, 5 * 64], f32, name="S")          # [p, t, grp]
            attn = mp.tile([P, 5 * 64], f32, name="attn")    # [p, t, grp]
            attnb = mp.tile([P, 5 * 64], bf16, name="attnb")
            mx = mp.tile([P, 64], f32, name="mx")
            zs = mp.tile([P, 64], f32, name="zs")
            dmin = mp.tile([P, 64], f32, name="dmin")
            mxp = mp.tile([P, 64], f32, name="mxp")
            tk = mp.tile([P, 256], f32, name="tk")
            o = mp.tile([P, 8 * KH], bf16, name="o")
            opad = [mp.tile([P, PHW + 2], bf16, name=f"opad{i}") for i in range(4)]
            y1 = [mp.tile([P, HW], f32, name=f"y1_{i}") for i in range(2)]
            sqbuf = mp.tile([P, HW], f32, name="sqbuf")
            h1p = [mp.tile([P, PHW + 2], bf16, name=f"h1p{i}") for i in range(2)]
            stc = mp.tile([P, 12], f32, name="stc")  # [(stat,co), chunk(3)]
            st = mp.tile([P, 4], f32, name="st")     # [sum0,sum1,sq0,sq1]
            gsum = mp.tile([P, 4], f32, name="gsum")
            ones1 = mp.tile([1, P], bf16, name="ones1")
            ones5 = mp.tile([1, 512], bf16, name="ones5")
            ident = mp.tile([P, P], bf16, name="ident")
            # per-layer conv weights (single-buffered; DMA overlaps attention)
            w1s = mp.tile([P, 72 * P], bf16, name="w1s")
            w2s = mp.tile([P, 36 * P], bf16, name="w2s")
            # per-layer consts (reloaded each layer)
            bng2 = mp.tile([P, 2], f32, name="bng2")
            bnb2 = mp.tile([P, 2], f32, name="bnb2")
            ob2bt = [mp.tile([1, P], bf16, name=f"ob2bt{i}") for i in range(2)]
            gamt = mp.tile([P, 1], f32, name="gamt")
            # BN scratch (both co at once)
            t1p = mp.tile([P, 2], f32, name="t1p")
            vartp = mp.tile([P, 2], f32, name="vartp")
            sqp = mp.tile([P, 2], f32, name="sqp")
            stdtp = mp.tile([P, 2], f32, name="stdtp")
            Acp = mp.tile([P, 2], f32, name="Acp")
            Bcp = mp.tile([P, 2], f32, name="Bcp")

            # init
            for i in range(2):
                nc.sync.dma_start(x[i][:], xin[i * P:(i + 1) * P, :])
                nc.scalar.copy(xb[i][:], x[i][:])
            for i in range(4):
                nc.vector.memset(opad[i][:], 0)
            for i in range(2):
                nc.vector.memset(h1p[i][:], 0)
            nc.vector.memset(ones1[:], 1.0)
            nc.vector.memset(ones5[:], 1.0)
            nc.sync.dma_start(ident[:], ident_d)
            nc.vector.memset(S[:], 0)
            nc.vector.memset(attn[:], 0)
            if not no_cc:
                # warmup collective: absorbs first-sync init/skew off-path
                nc.vector.memset(stc[:], 0)
                wui = dramp.tile([1, 4], f32, name="wui")
                wuo = dramp.tile([1, 4], f32, name="wuo", addr_space="Shared")
                nc.sync.dma_start(wui[0], stc[0, 0:4])
                nc.gpsimd.collective_compute(
                    "AllReduce", OP.add, replica_groups=[list(range(ncores))],
                    ins=[wui.opt()], outs=[wuo.opt()])

            S3t = S[:].rearrange("p (t g) -> p t g", g=64)
            at3t = attn[:].rearrange("p (t g) -> p t g", g=64)

            for l in range(layers):
                R = l + 1      # number of real keys
                T = R + 1      # +1 zero key

                # ---- per-layer consts + conv weight preload ----
                nc.sync.dma_start(w1s[:], w1d[l])
                nc.sync.dma_start(w2s[:], w2d[l])
                nc.sync.dma_start(bng2[:], bng2d[l])
                nc.sync.dma_start(bnb2[:], bnb2d[l])
                for i in range(2):
                    nc.sync.dma_start(ob2bt[i][:], ob2bd[l, i])
                nc.sync.dma_start(gamt[:], gamd[l])

                # ---- Q/K/V 1x1 convs, position-major (q first) ----
                for name, wdr, bdr, dest in (
                    ("q", wq, bq, qbt[:]),
                    ("k", wk, bk, kbt[l][:]),
                    ("v", wv, bv, vbt[l][:]),
                ):
                    bt = biasp.tile([1, KH], bf16, name=f"bias_{name}_{l}", tag="bias")
                    nc.sync.dma_start(bt[:], bdr[l])
                    wts = []
                    for ct in range(2):
                        wt = wkvp.tile([P, KH], bf16, name=f"w_{name}_{l}_{ct}", tag="wkv")
                        nc.sync.dma_start(wt[:], wdr[l, ct])
                        wts.append(wt)
                    for pb in range(8):
                        ps = kqvps.tile([P, KH], f32, name="kqv_ps")
                        nc.tensor.matmul(ps[:], ones1[:], bt[:], start=True, stop=False)
                        nc.tensor.matmul(ps[:], xb[0][:, pb * P:(pb + 1) * P], wts[0][:],
                                         start=False, stop=False)
                        nc.tensor.matmul(ps[:], xb[1][:, pb * P:(pb + 1) * P], wts[1][:],
                                         start=False, stop=True)
                        nc.scalar.copy(dest[:, pb * KH:(pb + 1) * KH], ps[:])

                # ---- scores: products + 3-step halving tree + reduce ----
                if stages < 2: continue
                for t in range(R):
                    pr = prodp.tile([P, 8 * KH], bf16, name="prodb")
                    pr3 = pr[:].rearrange("p (g d) -> p g d", d=KD)
                    nc.vector.tensor_mul(pr[:], qbt[:], kbt[t][:])
                    nc.vector.tensor_add(pr3[:, :, 0:32], pr3[:, :, 0:32],
                                         pr3[:, :, 32:64])
                    nc.vector.tensor_add(pr3[:, :, 0:16], pr3[:, :, 0:16],
                                         pr3[:, :, 16:32])
                    nc.vector.tensor_add(pr3[:, :, 0:8], pr3[:, :, 0:8],
                                         pr3[:, :, 8:16])
                    nc.vector.tensor_reduce(
                        out=S[:, t * 64:(t + 1) * 64], in_=pr3[:, :, 0:8],
                        axis=AX.X, op=OP.add)
                nc.vector.memset(S[:, R * 64:(R + 1) * 64], 0)  # zero key

                # ---- softmax over T slots (t-major; dense broadcasts) ----
                # layer 0: softmax over [s, 0] is exactly sigmoid(s)
                if stages < 3: continue
                if False:
                    nc.scalar.activation(attn[:, 0:64], S[:, 0:64], ACTF.Sigmoid)
                else:
                    nc.vector.tensor_reduce(
                        out=mx[:],
                        in_=S[:, 0:T * 64].rearrange("p (t g) -> p g t", g=64),
                        axis=AX.X, op=OP.max)
                    nc.vector.tensor_tensor(
                        at3t[:, 0:T, :], S3t[:, 0:T, :],
                        mx[:].unsqueeze(1).broadcast_to([P, T, 64]), OP.subtract)
                    nc.scalar.activation(attn[:, 0:T * 64], attn[:, 0:T * 64],
                                         ACTF.Exp)
                    nc.vector.tensor_reduce(
                        out=zs[:],
                        in_=attn[:, 0:T * 64].rearrange("p (t g) -> p g t", g=64),
                        axis=AX.X, op=OP.add)
                    nc.vector.reciprocal(zs[:], zs[:])
                    nc.vector.tensor_tensor(
                        at3t[:, 0:T, :], at3t[:, 0:T, :],
                        zs[:].unsqueeze(1).broadcast_to([P, T, 64]), OP.mult)

                # ---- sparse top-k (only T=5): 2nd smallest of 5 ----
                if T > TOPK:
                    a = [at3t[:, i, :] for i in range(5)]
                    tk4 = tk[:].rearrange("p (i g) -> p i g", g=64)
                    nc.vector.tensor_tensor(tk4[:, 0], a[0], a[1], OP.min)   # b0
                    nc.vector.tensor_tensor(tk4[:, 1], a[0], a[1], OP.max)   # b1
                    nc.vector.tensor_tensor(tk4[:, 2], a[2], a[3], OP.min)   # b2
                    nc.vector.tensor_tensor(tk4[:, 3], a[2], a[3], OP.max)   # b3
                    nc.vector.tensor_tensor(mx[:], tk4[:, 0], tk4[:, 2], OP.min)  # c0
                    nc.vector.tensor_tensor(mxp[:], tk4[:, 0], tk4[:, 2], OP.max)  # c1
                    nc.vector.tensor_tensor(zs[:], tk4[:, 1], tk4[:, 3], OP.min)   # d
                    nc.vector.tensor_tensor(mxp[:], mxp[:], zs[:], OP.min)   # s4
                    nc.vector.tensor_tensor(mx[:], mx[:], a[4], OP.max)      # e
                    nc.vector.tensor_tensor(dmin[:], mx[:], mxp[:], OP.min)  # delta
                    nc.vector.tensor_scalar_add(dmin[:], dmin[:], EPS)
                    nc.vector.tensor_tensor(
                        at3t[:, 0:T, :], at3t[:, 0:T, :],
                        dmin[:].unsqueeze(1).broadcast_to([P, T, 64]), OP.subtract)
                    nc.vector.tensor_scalar_max(attn[:, 0:T * 64], attn[:, 0:T * 64], 0.0)
                    nc.vector.tensor_reduce(
                        out=zs[:],
                        in_=attn[:, 0:T * 64].rearrange("p (t g) -> p g t", g=64),
                        axis=AX.X, op=OP.add)
                    nc.vector.tensor_scalar_add(zs[:], zs[:], EPS)
                    nc.vector.reciprocal(zs[:], zs[:])
                    nc.vector.tensor_tensor(
                        at3t[:, 0:T, :], at3t[:, 0:T, :],
                        zs[:].unsqueeze(1).broadcast_to([P, T, 64]), OP.mult)

                nc.vector.tensor_copy(attnb[:], attn[:])

                # ---- weighted sum in pb-halves (bf16, d-major) ----
                if stages < 4: continue
                CHUNKS = [(0, 15), (15, 15), (30, 2)]

                def wsum_half(half):
                    csl = slice(half * 4 * KH, (half + 1) * 4 * KH)
                    o4 = o[:, csl].rearrange("p (b d g) -> p b d g", d=KD, g=8)
                    for t in range(R):
                        v4 = vbt[t][:, csl].rearrange("p (b d g) -> p b d g",
                                                      d=KD, g=8)
                        ab = attnb[:, t * 64 + half * 32:t * 64 + half * 32 + 32
                                   ].rearrange("p (b g) -> p b g", g=8
                                               ).unsqueeze(2).broadcast_to(
                                       [P, 4, KD, 8])
                        if t == 0:
                            nc.vector.tensor_tensor(o4, v4, ab, OP.mult)
                        else:
                            tm = tmpp.tile([P, 4 * KH], bf16, name="wtmp")
                            tm4 = tm[:].rearrange("p (b d g) -> p b d g", d=KD, g=8)
                            nc.vector.tensor_tensor(tm4, v4, ab, OP.mult)
                            nc.vector.tensor_add(o[:, csl], o[:, csl], tm[:])

                def transposes(half):
                    for ht in range(4):
                        pst = trps.tile([P, 512], bf16, name="pst")
                        for k in range(4):
                            pb = half * 4 + k
                            nc.tensor.transpose(
                                pst[:, k * P:(k + 1) * P],
                                o[:, (pb * 4 + ht) * P:(pb * 4 + ht + 1) * P],
                                ident[:])
                        opv = opad[ht][:, 0:PHW].rearrange("c (i j) -> c i j", j=PW)
                        dst = opv[:, 1 + 16 * half:17 + 16 * half, 1:W + 1]
                        src = pst[:].rearrange("c (r j) -> c r j", j=W)
                        if (ht + half) % 2 == 0:
                            nc.vector.tensor_copy(dst, src)
                        else:
                            nc.scalar.copy(dst, src)

                def conv1_chunk(ic, i0, nr):
                    for co in range(2):
                        ps = convps.tile([P, 512], f32, name="c1ps", tag="cps")
                        nw = PW * nr
                        for tap in range(9):
                            ty, tx = tap // 3, tap % 3
                            for ci in range(4):
                                wi = (tap * 4 + ci) * 2 + co
                                base = PW * (i0 + ty) + tx
                                nc.tensor.matmul(
                                    ps[:, 0:nw], w1s[:, wi * P:(wi + 1) * P],
                                    opad[ci][:, base:base + nw],
                                    start=(tap == 0 and ci == 0),
                                    stop=(tap == 8 and ci == 3))
                        ych = y1[co][:, W * i0:W * (i0 + nr)]
                        nc.scalar.copy(
                            ych.rearrange("c (i j) -> c i j", j=W),
                            ps[:, 0:nw].rearrange("c (i j) -> c i j", j=PW)[:, :, 0:W])
                        # partial stats for this (chunk, co)
                        nc.vector.tensor_reduce(out=stc[:, co * 3 + ic:co * 3 + ic + 1],
                                                in_=ych, axis=AX.X, op=OP.add)
                        sch = sqbuf[:, W * i0:W * (i0 + nr)]
                        nc.scalar.square(sch, ych)
                        nc.vector.tensor_reduce(
                            out=stc[:, 6 + co * 3 + ic:7 + co * 3 + ic],
                            in_=sch, axis=AX.X, op=OP.add)

                wsum_half(0)
                wsum_half(1)
                transposes(0)
                conv1_chunk(0, *CHUNKS[0])
                transposes(1)
                conv1_chunk(1, *CHUNKS[1])
                conv1_chunk(2, *CHUNKS[2])

                # ---- BN stats combine + single AllReduce ----
                if stages < 7: continue
                nc.vector.tensor_reduce(
                    out=st[:], in_=stc[:].rearrange("p (q c) -> p q c", c=3),
                    axis=AX.X, op=OP.add)
                if no_cc:
                    nc.vector.tensor_scalar_mul(gsum[:], st[:], float(ncores))
                else:
                    cci = dramp.tile([1, 512], f32, name="cci", tag="cci")
                    cco = dramp.tile([1, 512], f32, name="cco", tag="cco",
                                     addr_space="Shared")
                    nc.sync.dma_start(cci[0].rearrange("(p j) -> p j", j=4), st[:])
                    nc.gpsimd.collective_compute(
                        "AllReduce", OP.add,
                        replica_groups=[list(range(ncores))],
                        ins=[cci.opt()], outs=[cco.opt()])
                    nc.sync.dma_start(gsum[:], cco[0].rearrange("(p j) -> p j", j=4))

                # ---- BN coefficients: A = g/sqrt(var+eps), B = b - mean*A ----
                if stages < 8: continue
                NTOT = float(ncores * HW)
                nc.vector.tensor_scalar_mul(t1p[:], gsum[:, 0:2], 1.0 / NTOT)
                nc.vector.tensor_scalar_mul(vartp[:], gsum[:, 2:4], 1.0 / NTOT)
                nc.vector.tensor_mul(sqp[:], t1p[:], t1p[:])
                nc.vector.tensor_sub(vartp[:], vartp[:], sqp[:])
                nc.vector.tensor_scalar_add(vartp[:], vartp[:], BN_EPS)
                nc.scalar.activation(stdtp[:], vartp[:], ACTF.Sqrt)
                nc.vector.reciprocal(stdtp[:], stdtp[:])
                nc.vector.tensor_mul(Acp[:], bng2[:], stdtp[:])
                nc.vector.tensor_mul(sqp[:], t1p[:], Acp[:])
                nc.vector.tensor_sub(Bcp[:], bnb2[:], sqp[:])
                # h1 = relu(A*y1 + B) (row-split to unblock conv2 chunk 1)
                for (r0, nr2) in ((0, 17), (17, 15)):
                    for co in range(2):
                        h1v = h1p[co][:, 0:PHW].rearrange("c (i j) -> c i j", j=PW)
                        nc.scalar.activation(
                            h1v[:, 1 + r0:1 + r0 + nr2, 1:W + 1],
                            y1[co][:, W * r0:W * (r0 + nr2)].rearrange(
                                "c (i j) -> c i j", j=W),
                            ACTF.Relu, bias=Bcp[:, co:co + 1], scale=Acp[:, co:co + 1])

                # ---- conv3x3 #2 (bf16) + ob2 bias row + residual update ----
                if stages < 9: continue
                for co in range(2):
                    for (i0, nr) in CHUNKS:
                        ps = convps.tile([P, 512], f32, name="c2ps", tag="cps")
                        nw = PW * nr
                        nc.tensor.matmul(ps[:, 0:nw], ob2bt[co][:], ones5[:, 0:nw],
                                         start=True, stop=False)
                        for tap in range(9):
                            ty, tx = tap // 3, tap % 3
                            for ci in range(2):
                                wi = (tap * 2 + ci) * 2 + co
                                base = PW * (i0 + ty) + tx
                                nc.tensor.matmul(
                                    ps[:, 0:nw], w2s[:, wi * P:(wi + 1) * P],
                                    h1p[ci][:, base:base + nw],
                                    start=False, stop=(tap == 8 and ci == 1))
                        xslice = x[co][:, W * i0:W * (i0 + nr)]
                        nc.vector.scalar_tensor_tensor(
                            out=xslice.rearrange("c (i j) -> c i j", j=W),
                            in0=ps[:, 0:nw].rearrange("c (i j) -> c i j", j=PW)[:, :, 0:W],
                            scalar=gamt[:],
                            in1=xslice.rearrange("c (i j) -> c i j", j=W),
                            op0=OP.mult, op1=OP.add)
                    if l < layers - 1:
                        nc.scalar.copy(xb[co][:], x[co][:])
                    else:
                        nc.sync.dma_start(out[co * P:(co + 1) * P, :], x[co][:])

    nc.compile()
    return nc


def _host_prep(inputs):
    bf = ml_dtypes.bfloat16
    kw, kb, qw, qb = inputs["kw"], inputs["kb"], inputs["qw"], inputs["qb"]
    vw, vb = inputs["vw"], inputs["vb"]
    ow1, ow2 = inputs["ow1"], inputs["ow2"]
    gammas, ob2 = inputs["gammas"], inputs["ob2"]

    # d-major channel permutation for V / O / conv1-input
    perm2 = np.arange(512).reshape(NH, KD).T.flatten()  # new (d,g) <- old (g,d)
    vw = vw[:, perm2, :]
    vb = vb[:, perm2]
    ow1 = ow1[:, :, perm2, :, :]

    def packw(wm):  # [L, KH, C] -> [L, 2, 128, KH]
        return np.ascontiguousarray(
            wm.transpose(0, 2, 1).reshape(L, 2, P, KH)).astype(bf)

    d = {}
    d["wq"] = packw(qw / 8.0)
    d["wk"] = packw(kw)
    d["wv"] = packw(vw)
    d["bq"] = np.ascontiguousarray((qb / 8.0).reshape(L, 1, KH)).astype(bf)
    d["bk"] = np.ascontiguousarray(kb.reshape(L, 1, KH)).astype(bf)
    d["bv"] = np.ascontiguousarray(vb.reshape(L, 1, KH)).astype(bf)
    # ow1 [L, 256, 512, 3, 3] -> [L, tap, ci(4), co(2), a(cin128), b(cout128)]
    # -> partition-major [L, a, tap, ci, co, b] for one contiguous DMA/layer
    a1 = ow1.reshape(L, 2, P, 4, P, 3, 3).transpose(0, 5, 6, 3, 1, 4, 2)
    a1 = a1.reshape(L, 9, 4, 2, P, P).transpose(0, 4, 1, 2, 3, 5)
    d["w1d"] = np.ascontiguousarray(a1.reshape(L, P, 72 * P)).astype(bf)
    a2 = ow2.reshape(L, 2, P, 2, P, 3, 3).transpose(0, 5, 6, 3, 1, 4, 2)
    a2 = a2.reshape(L, 9, 2, 2, P, P).transpose(0, 4, 1, 2, 3, 5)
    d["w2d"] = np.ascontiguousarray(a2.reshape(L, P, 36 * P)).astype(bf)
    d["ob2bd"] = np.ascontiguousarray(ob2.reshape(L, 2, 1, P)).astype(bf)
    d["ident"] = np.eye(P, dtype=np.float32).astype(bf)
    d["bng2d"] = np.ascontiguousarray(
        inputs["bn_g"].reshape(L, 2, P).transpose(0, 2, 1)).astype(np.float32)
    d["bnb2d"] = np.ascontiguousarray(
        inputs["bn_b"].reshape(L, 2, P).transpose(0, 2, 1)).astype(np.float32)
    d["gamd"] = np.ascontiguousarray(
        np.broadcast_to(gammas[:, None, None], (L, P, 1))).astype(np.float32)
    return d


def kernel(**inputs):
    if "nc" not in _compiled:
        _compiled["nc"] = _build()
    nc = _compiled["nc"]
    shared = _host_prep(inputs)
    x = np.ascontiguousarray(inputs["x"].reshape(B, C, HW)).astype(np.float32)
    in_maps = []
    for c in range(NC):
        m = dict(shared)
        m["xin"] = x[c]
        in_maps.append(m)
    res = bass_utils.run_bass_kernel_spmd(nc, in_maps, core_ids=list(range(NC)))
    outs = np.stack([res.results[c]["out"] for c in range(NC)])
    return outs.reshape(B, C, H, W).astype(np.float32)
